# revision 3
# baseline (speedup 1.0000x reference)
"""v3: Strassen-1 token-parallel GPTQ linear (fp16, 8-core SPMD).

Per core (1024 tokens): C = A @ B with A = x-shard [1024, 4096],
B = W [4096, 4096]. 2x2 block split: tokens (T1/T2), k (K1/K2), n (N1/N2).
Strassen: 7 products Mi = Si @ Ti of [512 tok, 2048 k, 2048 n] instead of 8
=> 1792 instead of 2048 128x128x512 matmuls per iteration (-12.5% PE).

 - S-combos (A-side) and T-combos (B-side) are computed on HOST; S ships
   once (resident, 14 MiB SBUF), T streams per n-tile chunk [128,7,16,128]
   (3.5 MiB x 16 = 56 MiB/iter) on the Pool DGE queue.
 - Per n-tile: 7 psum accumulation chains (16 MMs each) -> ACT copy to
   fp16 M-staging -> DVE recombines (8 tensor-ops) into C11/C12/C21/C22
   fp16 tiles -> output DMA on SP queue.
"""
import numpy as np

import concourse.bass as bass
import concourse.tile as tile
import concourse.mybir as mybir
from concourse import bacc
from concourse.bass_utils import run_bass_kernel_spmd

NCORES = 8
B, SEQ, IN_F, OUT_F = 4, 2048, 4096, 4096
GS = 128
NG = IN_F // GS
M_TOT = B * SEQ
M = M_TOT // NCORES       # 1024 tokens/core
HK = IN_F // 2            # 2048, k half
HN = OUT_F // 2           # 2048, n half
HT = M // 2               # 512, token half
KT = HK // 128            # 16 k-tiles per half
NTILES = HN // 128        # 16 n-tiles per half
F16 = mybir.dt.float16
F32 = mybir.dt.float32

_cache = {}


def _build(m=M, iters=1):
    assert m == M
    nc = bacc.Bacc("TRN2", target_bir_lowering=False, debug=False,
                   num_devices=NCORES)
    s_in = nc.dram_tensor("s", [128, 7 * KT * HT], F16,
                          kind="ExternalInput").ap()
    wd = nc.dram_tensor("wd", [NTILES * 128, 7 * KT * 128], F16,
                        kind="ExternalInput").ap()
    outT = nc.dram_tensor("outT", [OUT_F, m], F16, kind="ExternalOutput").ap()

    with tile.TileContext(nc) as tc:
        with tc.tile_pool(name="resident", bufs=1) as res:
            s_sb = res.tile([128, 7, KT, HT], F16)
            nc.sync.dma_start(s_sb[:], s_in)

            from contextlib import ExitStack
            _loop = ExitStack()
            if iters > 1:
                _loop.enter_context(tc.For_i(0, iters, 1))
            with tc.tile_pool(name="wf", bufs=2) as wfp, \
                 tc.tile_pool(name="mst", bufs=2) as mpool, \
                 tc.tile_pool(name="u", bufs=2) as upool, \
                 tc.tile_pool(name="cst", bufs=8) as cpool, \
                 tc.tile_pool(name="ps", bufs=1, space="PSUM") as psp:
                for nt in range(NTILES):
                    w_t = wfp.tile([128, 7, KT, 128], F16, tag="w",
                                   name=f"w_{nt}")
                    # alternate weight DMAs across Pool/ACT DGE queues so
                    # they never serialize behind each other or the SP-queue
                    # output DMAs (measured ~15% effect in contended windows)
                    weng = nc.gpsimd if nt % 2 == 0 else nc.scalar
                    weng.dma_start(w_t[:], wd[nt * 128:(nt + 1) * 128, :])
                    ms = []
                    for i in range(7):
                        ps = psp.tile([128, HT], F32, tag=f"ps{i}",
                                      name=f"ps_{nt}_{i}")
                        for kt in range(KT):
                            nc.tensor.matmul(
                                ps[:], w_t[:, i, kt, :], s_sb[:, i, kt, :],
                                start=(kt == 0), stop=(kt == KT - 1))
                        mt = mpool.tile([128, HT], F16, tag=f"m{i}",
                                        name=f"m_{nt}_{i}")
                        nc.scalar.copy(mt[:], ps[:])
                        ms.append(mt)
                    m1, m2, m3, m4, m5, m6, m7 = ms

                    def cst(tag):
                        return cpool.tile([128, HT], F16, tag="c",
                                          name=f"c_{nt}_{tag}")

                    # C11 = M1 + M4 - M5 + M7
                    u1 = upool.tile([128, HT], F16, tag="u1",
                                    name=f"u1_{nt}")
                    nc.vector.tensor_add(u1[:], m1[:], m4[:])
                    u2 = upool.tile([128, HT], F16, tag="u2",
                                    name=f"u2_{nt}")
                    nc.vector.tensor_sub(u2[:], u1[:], m5[:])
                    c11 = cst("c11")
                    nc.vector.tensor_add(c11[:], u2[:], m7[:])
                    nc.sync.dma_start(
                        outT[nt * 128:(nt + 1) * 128, 0:HT], c11[:])
                    # C12 = M3 + M5
                    c12 = cst("c12")
                    nc.vector.tensor_add(c12[:], m3[:], m5[:])
                    nc.sync.dma_start(
                        outT[HN + nt * 128:HN + (nt + 1) * 128, 0:HT],
                        c12[:])
                    # C21 = M2 + M4
                    c21 = cst("c21")
                    nc.vector.tensor_add(c21[:], m2[:], m4[:])
                    nc.sync.dma_start(
                        outT[nt * 128:(nt + 1) * 128, HT:m], c21[:])
                    # C22 = M1 - M2 + M3 + M6
                    u3 = upool.tile([128, HT], F16, tag="u3",
                                    name=f"u3_{nt}")
                    nc.vector.tensor_sub(u3[:], m1[:], m2[:])
                    u4 = upool.tile([128, HT], F16, tag="u4",
                                    name=f"u4_{nt}")
                    nc.vector.tensor_add(u4[:], u3[:], m3[:])
                    c22 = cst("c22")
                    nc.vector.tensor_add(c22[:], u4[:], m6[:])
                    nc.sync.dma_start(
                        outT[HN + nt * 128:HN + (nt + 1) * 128, HT:m],
                        c22[:])
            _loop.close()
    nc.compile()
    return nc


def _dequant_w(qweight, qzeros, scales):
    u = qweight.view(np.uint32)
    shifts = (4 * np.arange(8, dtype=np.uint32))[None, :, None]
    q = ((u[:, None, :] >> shifts) & np.uint32(0xF)).reshape(IN_F, OUT_F)
    uz = qzeros.view(np.uint32)
    shz = (4 * np.arange(8, dtype=np.uint32))[None, None, :]
    z = ((uz[:, :, None] >> shz) & np.uint32(0xF)).reshape(NG, OUT_F)
    return ((q.astype(np.float32).reshape(NG, GS, OUT_F)
             - (z.astype(np.float32) + 1.0)[:, None, :])
            * scales[:, None, :]).reshape(IN_F, OUT_F)


def _prep(x, qweight, qzeros, scales, m=M, ncores=NCORES):
    w = _dequant_w(qweight, qzeros, scales)

    b11 = w[:HK, :HN]
    b12 = w[:HK, HN:]
    b21 = w[HK:, :HN]
    b22 = w[HK:, HN:]
    tcombos = np.stack([
        b11 + b22, b11, b12 - b22, b21 - b11, b22, b11 + b12, b21 + b22,
    ]).astype(np.float16)                     # [7, 2048 k', 2048 n']
    # -> [nt', p, i, kt', n'']
    wd = np.ascontiguousarray(
        tcombos.reshape(7, KT, 128, NTILES, 128)
        .transpose(3, 2, 0, 1, 4).reshape(NTILES * 128, 7 * KT * 128))

    xs_all = x.reshape(M_TOT, IN_F)
    in_maps = []
    for c in range(ncores):
        xs = xs_all[c * m:(c + 1) * m]
        a11 = xs[:HT, :HK]
        a12 = xs[:HT, HK:]
        a21 = xs[HT:, :HK]
        a22 = xs[HT:, HK:]
        scombos = np.stack([
            a11 + a22, a21 + a22, a11, a22, a11 + a12, a21 - a11, a12 - a22,
        ]).astype(np.float16)                 # [7, 512 t, 2048 k']
        s = np.ascontiguousarray(
            scombos.reshape(7, HT, KT, 128)
            .transpose(3, 0, 2, 1).reshape(128, 7 * KT * HT))
        in_maps.append({"s": s, "wd": wd})
    return in_maps


def kernel(x, qweight, qzeros, scales):
    x = np.ascontiguousarray(np.asarray(x, dtype=np.float32))
    qweight = np.ascontiguousarray(np.asarray(qweight, dtype=np.int32))
    qzeros = np.ascontiguousarray(np.asarray(qzeros, dtype=np.int32))
    scales = np.ascontiguousarray(np.asarray(scales, dtype=np.float32))
    if "nc" not in _cache:
        _cache["nc"] = _build()
    nc = _cache["nc"]
    in_maps = _prep(x, qweight, qzeros, scales)
    results = run_bass_kernel_spmd(
        nc, in_maps, core_ids=list(range(NCORES))).results
    outs = [r["outT"] for r in results]
    full = np.concatenate(outs, axis=1)
    return np.ascontiguousarray(full.T).reshape(B, SEQ, OUT_F).astype(np.float32)


# revision 4
# speedup vs baseline: 1.4524x; 1.4524x over previous
"""v4: Strassen-1 with on-chip B-combos (fp16, 8-core SPMD token-parallel).

Like v3 (7 products of [512,2048,2048] per core, host-side S-combos resident)
but the B-side streams RAW quadrant column-slices (32 MiB/iter, same as the
direct kernel) and DVE builds the 5 non-trivial T-combos per n-tile chunk:
  T1=B11+B22  T3=B12-B22  T4=B21-B11  T6=B11+B12  T7=B21+B22
T2=B11 and T5=B22 are read directly from the raw tile.
Weight DMA on Pool queue; outputs on SP queue.
"""
import numpy as np

import concourse.bass as bass
import concourse.tile as tile
import concourse.mybir as mybir
from concourse import bacc
from concourse.bass_utils import run_bass_kernel_spmd

NCORES = 8
B, SEQ, IN_F, OUT_F = 4, 2048, 4096, 4096
GS = 128
NG = IN_F // GS
M_TOT = B * SEQ
M = M_TOT // NCORES
HK = IN_F // 2            # 2048
HN = OUT_F // 2           # 2048
HT = M // 2               # 512
KT = HK // 128            # 16 k-tiles per half
NT_ALL = IN_F // 128      # 32
NTILES = HN // 128        # 16 n-tiles per half
F16 = mybir.dt.float16
F32 = mybir.dt.float32

_cache = {}


def _build(m=M, iters=1):
    assert m == M
    nc = bacc.Bacc("TRN2", target_bir_lowering=False, debug=False,
                   num_devices=NCORES)
    s_in = nc.dram_tensor("s", [128, 7 * KT * HT], F16,
                          kind="ExternalInput").ap()
    wd = nc.dram_tensor("wd", [NTILES * 128, NT_ALL * 2 * 128], F16,
                        kind="ExternalInput").ap()
    outT = nc.dram_tensor("outT", [OUT_F, m], F16, kind="ExternalOutput").ap()

    with tile.TileContext(nc) as tc:
        with tc.tile_pool(name="resident", bufs=1) as res:
            s_sb = res.tile([128, 7, KT, HT], F16)
            nc.sync.dma_start(s_sb[:], s_in)

            from contextlib import ExitStack
            _loop = ExitStack()
            if iters > 1:
                _loop.enter_context(tc.For_i(0, iters, 1))
            with tc.tile_pool(name="wf", bufs=2) as wfp, \
                 tc.tile_pool(name="tc7", bufs=2) as tcp, \
                 tc.tile_pool(name="mst", bufs=2) as mpool, \
                 tc.tile_pool(name="u", bufs=1) as upool, \
                 tc.tile_pool(name="cst", bufs=4) as cpool, \
                 tc.tile_pool(name="ps", bufs=1, space="PSUM") as psp:
                for nt in range(NTILES):
                    # raw quadrant slices: [128, kt(32), h(2), 128]
                    w_t = wfp.tile([128, NT_ALL, 2, 128], F16, tag="w",
                                   name=f"w_{nt}")
                    weng = nc.gpsimd if nt % 2 == 0 else nc.scalar
                    weng.dma_start(w_t[:], wd[nt * 128:(nt + 1) * 128, :])
                    # on-chip combos: [128, 5, KT, 128] fp16
                    # order: [T1, T3, T4, T6, T7]
                    t_c = tcp.tile([128, 5, KT, 128], F16, tag="t",
                                   name=f"t_{nt}")
                    b11 = w_t[:, 0:KT, 0, :]
                    b12 = w_t[:, 0:KT, 1, :]
                    b21 = w_t[:, KT:NT_ALL, 0, :]
                    b22 = w_t[:, KT:NT_ALL, 1, :]
                    nc.vector.tensor_add(t_c[:, 0], b11, b22)   # T1
                    nc.vector.tensor_sub(t_c[:, 1], b12, b22)   # T3
                    nc.vector.tensor_sub(t_c[:, 2], b21, b11)   # T4
                    nc.vector.tensor_add(t_c[:, 3], b11, b12)   # T6
                    nc.vector.tensor_add(t_c[:, 4], b21, b22)   # T7

                    def lhsT(i, kt):
                        # product index i (0-based M1..M7) -> weight slice
                        if i == 1:                       # T2 = B11
                            return w_t[:, kt, 0, :]
                        if i == 4:                       # T5 = B22
                            return w_t[:, KT + kt, 1, :]
                        ci = {0: 0, 2: 1, 3: 2, 5: 3, 6: 4}[i]
                        return t_c[:, ci, kt, :]

                    ms = []
                    for i in range(7):
                        ps = psp.tile([128, HT], F32, tag=f"ps{i}",
                                      name=f"ps_{nt}_{i}")
                        for kt in range(KT):
                            nc.tensor.matmul(
                                ps[:], lhsT(i, kt), s_sb[:, i, kt, :],
                                start=(kt == 0), stop=(kt == KT - 1))
                        mt = mpool.tile([128, HT], F16, tag=f"m{i}",
                                        name=f"m_{nt}_{i}")
                        nc.scalar.copy(mt[:], ps[:])
                        ms.append(mt)
                    m1, m2, m3, m4, m5, m6, m7 = ms

                    u1 = upool.tile([128, HT], F16, tag="u1", name=f"u1_{nt}")
                    nc.vector.tensor_add(u1[:], m1[:], m4[:])
                    u2 = upool.tile([128, HT], F16, tag="u2", name=f"u2_{nt}")
                    nc.vector.tensor_sub(u2[:], u1[:], m5[:])
                    c11 = cpool.tile([128, HT], F16, tag="c", name=f"ca_{nt}")
                    nc.vector.tensor_add(c11[:], u2[:], m7[:])
                    nc.sync.dma_start(outT[nt * 128:(nt + 1) * 128, 0:HT],
                                      c11[:])
                    c12 = cpool.tile([128, HT], F16, tag="c", name=f"cb_{nt}")
                    nc.vector.tensor_add(c12[:], m3[:], m5[:])
                    nc.sync.dma_start(
                        outT[HN + nt * 128:HN + (nt + 1) * 128, 0:HT], c12[:])
                    c21 = cpool.tile([128, HT], F16, tag="c", name=f"cc_{nt}")
                    nc.vector.tensor_add(c21[:], m2[:], m4[:])
                    nc.sync.dma_start(outT[nt * 128:(nt + 1) * 128, HT:m],
                                      c21[:])
                    u3 = upool.tile([128, HT], F16, tag="u3", name=f"u3_{nt}")
                    nc.vector.tensor_sub(u3[:], m1[:], m2[:])
                    u4 = upool.tile([128, HT], F16, tag="u4", name=f"u4_{nt}")
                    nc.vector.tensor_add(u4[:], u3[:], m3[:])
                    c22 = cpool.tile([128, HT], F16, tag="c", name=f"cd_{nt}")
                    nc.vector.tensor_add(c22[:], u4[:], m6[:])
                    nc.sync.dma_start(
                        outT[HN + nt * 128:HN + (nt + 1) * 128, HT:m],
                        c22[:])
            _loop.close()
    nc.compile()
    return nc


def _dequant_w(qweight, qzeros, scales):
    u = qweight.view(np.uint32)
    shifts = (4 * np.arange(8, dtype=np.uint32))[None, :, None]
    q = ((u[:, None, :] >> shifts) & np.uint32(0xF)).reshape(IN_F, OUT_F)
    uz = qzeros.view(np.uint32)
    shz = (4 * np.arange(8, dtype=np.uint32))[None, None, :]
    z = ((uz[:, :, None] >> shz) & np.uint32(0xF)).reshape(NG, OUT_F)
    return ((q.astype(np.float32).reshape(NG, GS, OUT_F)
             - (z.astype(np.float32) + 1.0)[:, None, :])
            * scales[:, None, :]).reshape(IN_F, OUT_F)


def _prep(x, qweight, qzeros, scales, m=M, ncores=NCORES):
    w16 = _dequant_w(qweight, qzeros, scales).astype(np.float16)
    # raw layout: wd[nt*128+p, kt*256 + h*128 + n''] = w16[kt*128+p, h*HN + nt*128 + n'']
    wd = np.ascontiguousarray(
        w16.reshape(NT_ALL, 128, 2, NTILES, 128)   # [kt, p, h, nt, n'']
        .transpose(3, 1, 0, 2, 4)                  # [nt, p, kt, h, n'']
        .reshape(NTILES * 128, NT_ALL * 2 * 128))

    xs_all = x.reshape(M_TOT, IN_F)
    in_maps = []
    for c in range(ncores):
        xs = xs_all[c * m:(c + 1) * m]
        a11 = xs[:HT, :HK]
        a12 = xs[:HT, HK:]
        a21 = xs[HT:, :HK]
        a22 = xs[HT:, HK:]
        scombos = np.stack([
            a11 + a22, a21 + a22, a11, a22, a11 + a12, a21 - a11, a12 - a22,
        ]).astype(np.float16)
        s = np.ascontiguousarray(
            scombos.reshape(7, HT, KT, 128)
            .transpose(3, 0, 2, 1).reshape(128, 7 * KT * HT))
        in_maps.append({"s": s, "wd": wd})
    return in_maps


def kernel(x, qweight, qzeros, scales):
    x = np.ascontiguousarray(np.asarray(x, dtype=np.float32))
    qweight = np.ascontiguousarray(np.asarray(qweight, dtype=np.int32))
    qzeros = np.ascontiguousarray(np.asarray(qzeros, dtype=np.int32))
    scales = np.ascontiguousarray(np.asarray(scales, dtype=np.float32))
    if "nc" not in _cache:
        _cache["nc"] = _build()
    nc = _cache["nc"]
    in_maps = _prep(x, qweight, qzeros, scales)
    results = run_bass_kernel_spmd(
        nc, in_maps, core_ids=list(range(NCORES))).results
    outs = [r["outT"] for r in results]
    full = np.concatenate(outs, axis=1)
    return np.ascontiguousarray(full.T).reshape(B, SEQ, OUT_F).astype(np.float32)
